# revision 30
# baseline (speedup 1.0000x reference)
"""CKConv (SIREN continuous-kernel conv) Trainium2 Bass kernel.

Math: the reference evaluates a SIREN net at rel[e,s] = t[s] - t_eval[e],
masks causally (rel <= 0), and contracts with x:
    out[e,g] = sum_{s<=e, c} K(rel[e,s])[g,c] * x[s,c]
Both t and t_eval are arange(512)/512, so rel[e,s] = (s-e)/512 exactly in
fp32 -- it depends only on the lag j = e - s in [0, 511].  The net therefore
only needs evaluation at 512 distinct inputs rel_j = -j/512, and the output
is a causal Toeplitz conv:
    out[e] = sum_{j=0}^{e} K'[j] @ x[e-j],   K'[j] in R^{16x16}.

Sharding: 8 cores split the contraction by (input channel c, lag block jb):
core m owns channels {2m, 2m+1} x all 4 lag blocks of 128.  Each core:
  1. evaluates the SIREN net at all 512 lags for its 2 channels' 32 (g,c)
     outputs -> K table (128 j x 32) per lag block, j on partitions
  2. runs 8 PSUM-accumulated matmuls K-block^T @ Hankel(x) -> partial (16,512)
Host builds the Hankel tiles H[p,e] = xpad[e - 128*jb - p, c] (pure data
movement of x), sums the 8 per-core partials and transposes -> (512, 16).

sin(x) is computed with explicit range reduction so the ACT table only ever
sees arguments in [-pi, pi] (DVE has no fp mod; use the magic-number
round-to-nearest trick, pure add/sub):
    u = arg/(2pi) ; k = (u + 1.5*2^23) - 1.5*2^23 ; sin(arg) = Sin(2pi*(u-k))
"""

import numpy as np

import concourse.bass as bass
import concourse.mybir as mybir
import concourse.tile as tile
from concourse import bacc
from concourse.bass_utils import run_bass_kernel_spmd

F32 = mybir.dt.float32
L = 512          # sequence length == L_eval
CIN = 16
COUT = 16
H = 32           # SIREN hidden
OMEGA = 32.5
NCORES = 8
NJB = 4          # lag blocks of 128
PAD = 512        # zero padding rows in front of x for the Hankel build
TWO_PI = 2.0 * np.pi
MAGIC = float(1.5 * 2.0**23)  # fp32 add/sub rounds to nearest integer

_CACHE = {}


def _build_module():
    # Bacc (not raw Bass): its compile() splits multi-sem sync waits into
    # event-semaphore instructions -- walrus allows only 1 wait per inst.
    nc = bacc.Bacc("TRN2", target_bir_lowering=False, debug=False)

    # One packed param tensor -> one DMA -> one sem wait for consumers.
    # layout (33, 608): [:32,0:512]=relrep, [:32,512]=a1, [:32,513]=c1,
    # [:32,514]=c2, [:32,515:547]=w2t, [:33,547:579]=w3b (rows for this
    # core's 32 (c,g) cols, c-major; last row = b3)
    params_d = nc.dram_tensor("params", [H + 1, 608], F32, kind="ExternalInput")
    # Hankel tiles packed along free dim: chunk k = 2*jb + ci at cols
    # [k*L, (k+1)*L); H_k[p, e] = xpad[e - 128*jb - p, c]
    hank_d = nc.dram_tensor("hank", [128, 2 * NJB * L], F32, kind="ExternalInput")
    out_d = nc.dram_tensor("out", [COUT, L], F32, kind="ExternalOutput")

    with tile.TileContext(nc) as tc:
        with (
            tc.tile_pool(name="sb", bufs=1) as sb,
            tc.tile_pool(name="ps", bufs=1, space="PSUM") as ps,
            tc.tile_pool(name="ps2", bufs=2, space="PSUM") as ps2,
        ):
            # Prewarm the Sin ACT table set so its ~2.7us load overlaps DMAs.
            warm = sb.tile([1, 1], F32)
            nc.vector.memset(warm[:], 0.0)
            nc.scalar.activation(warm[:], warm[:], mybir.ActivationFunctionType.Sin)

            pt = sb.tile([H + 1, 608], F32)
            nc.sync.dma_start(pt[:], params_d[:])
            relrep = pt[0:H, 0:L]
            a1 = pt[0:H, 512:513]
            c1 = pt[0:H, 513:514]
            c2 = pt[0:H, 514:515]
            ht = sb.tile([128, 2 * NJB * L], F32)
            nc.sync.dma_start(ht[:], hank_d[:])

            # Matmul operands must have the DVE as their last writer: walrus
            # here allows only ONE sync wait per fused LDWEIGHTS/Matmult, so
            # every matmul may depend on at most one semaphore.
            w2t = sb.tile([H, H], F32)
            nc.vector.tensor_copy(w2t[:], pt[0:H, 515:547])
            w3b = sb.tile([H + 1, 2 * COUT], F32)
            nc.vector.tensor_copy(w3b[:], pt[0 : H + 1, 547:579])

            # ---- SIREN layer 1: h1[i, j] = sin(OMEGA*(rel_j*W1[i] + b1[i]))
            # u = rel*A1 + C1 with A1 = OMEGA*W1/2pi, C1 = OMEGA*b1/2pi
            u1 = sb.tile([H, L], F32)
            nc.vector.tensor_scalar(
                u1[:], relrep, a1, c1,
                mybir.AluOpType.mult, mybir.AluOpType.add,
            )
            k1 = sb.tile([H, L], F32)
            nc.vector.tensor_scalar(
                k1[:], u1[:], MAGIC, MAGIC,
                mybir.AluOpType.add, mybir.AluOpType.subtract,
            )
            nc.vector.tensor_sub(u1[:], u1[:], k1[:])
            h1t = sb.tile([H, L], F32)
            nc.scalar.activation(
                h1t[:], u1[:], mybir.ActivationFunctionType.Sin, scale=TWO_PI
            )
            h1 = sb.tile([H, L], F32)
            nc.vector.tensor_copy(h1[:], h1t[:])

            # ---- SIREN layer 2: h2 = sin(OMEGA*(W2 @ h1 + b2)), o on partitions
            mm2 = ps.tile([H, L], F32)
            nc.tensor.matmul(mm2[:], w2t[:], h1[:], start=True, stop=True)
            u2 = sb.tile([H, L], F32)
            nc.vector.tensor_scalar(
                u2[:], mm2[:], float(OMEGA / TWO_PI), c2,
                mybir.AluOpType.mult, mybir.AluOpType.add,
            )
            k2 = sb.tile([H, L], F32)
            nc.vector.tensor_scalar(
                k2[:], u2[:], MAGIC, MAGIC,
                mybir.AluOpType.add, mybir.AluOpType.subtract,
            )
            nc.vector.tensor_sub(u2[:], u2[:], k2[:])
            h2t = sb.tile([H, L], F32)
            nc.scalar.activation(
                h2t[:], u2[:], mybir.ActivationFunctionType.Sin, scale=TWO_PI
            )
            h2 = sb.tile([H + 1, L], F32)
            nc.vector.memset(h2[H : H + 1, :], 1.0)  # ones row folds b3 into mm
            nc.vector.tensor_copy(h2[0:H, :], h2t[:])

            # ---- layer 3: K[j, (c,g)] = W3' @ h2 + b3, per lag block
            ksb = sb.tile([128, NJB * 2 * COUT], F32)
            for b in range(NJB):
                ktp = ps2.tile([128, 2 * COUT], F32)
                nc.tensor.matmul(
                    ktp[:], h2[:, b * 128 : (b + 1) * 128], w3b[:],
                    start=True, stop=True,
                )
                nc.vector.tensor_copy(
                    ksb[:, b * 2 * COUT : (b + 1) * 2 * COUT], ktp[:]
                )

            # ---- causal conv: accumulate 8 matmuls into one PSUM tile
            out_ps = ps.tile([COUT, L], F32)
            # Dummy matmul reading only ksb: absorbs the DVE(ksb) wait on PE
            # so the first conv matmul carries only the DMA(ht) wait (walrus
            # allows one sync wait per matmul).  Its output is overwritten by
            # the conv group's start=True, and the WAW dep pins the order.
            nc.tensor.matmul(
                out_ps[:, 0:1], ksb[:, 0:COUT], ksb[:, 0:1], start=True, stop=True
            )
            nmm = 2 * NJB
            k = 0
            for b in range(NJB):
                for ci in range(2):
                    # lhsT columns: this (jb, ci) block's 16 g-values
                    lhs = ksb[:, b * 2 * COUT + ci * COUT : b * 2 * COUT + (ci + 1) * COUT]
                    rhs = ht[:, (b * 2 + ci) * L : (b * 2 + ci + 1) * L]
                    nc.tensor.matmul(
                        out_ps[:], lhs, rhs, start=(k == 0), stop=(k == nmm - 1)
                    )
                    k += 1

            out_sb = sb.tile([COUT, L], F32)
            nc.vector.tensor_copy(out_sb[:], out_ps[:])
            nc.sync.dma_start(out_d[:], out_sb[:])

    nc.compile()
    return nc


def _host_prep(inputs):
    """Fold params and build per-core in_maps (all fp32 numpy)."""
    x = np.asarray(inputs["x"], np.float32)
    t = np.asarray(inputs["t"], np.float32)
    t_eval = np.asarray(inputs["t_eval"], np.float32)
    v1 = np.asarray(inputs["v1"], np.float32)
    g1 = np.asarray(inputs["g1"], np.float32)
    b1 = np.asarray(inputs["b1"], np.float32)
    v2 = np.asarray(inputs["v2"], np.float32)
    g2 = np.asarray(inputs["g2"], np.float32)
    b2 = np.asarray(inputs["b2"], np.float32)
    W3 = np.asarray(inputs["W3"], np.float32)
    b3 = np.asarray(inputs["b3"], np.float32)

    # weight norm (fp32, matching reference)
    W1 = (g1[:, None] * v1 / np.linalg.norm(v1, axis=1, keepdims=True))[:, 0]
    W2 = g2[:, None] * v2 / np.linalg.norm(v2, axis=1, keepdims=True)

    # rel_j = t[0] - t_eval[j]  (== -j/512 exactly on the arange grid)
    rel = (np.float32(t[0]) - t_eval).astype(np.float32)

    a1 = (np.float64(OMEGA) * W1.astype(np.float64) / TWO_PI).astype(np.float32)
    c1 = (np.float64(OMEGA) * b1.astype(np.float64) / TWO_PI).astype(np.float32)
    c2 = (np.float64(OMEGA) * b2.astype(np.float64) / TWO_PI).astype(np.float32)

    relrep = np.broadcast_to(rel, (H, L)).copy()

    xpad = np.zeros((PAD + L, CIN), np.float32)
    xpad[PAD:] = x

    in_maps = []
    for m in range(NCORES):
        cols = []
        for ci in range(2):
            c = 2 * m + ci
            cols.extend(g * CIN + c for g in range(COUT))
        w3b = np.concatenate(
            [W3[cols, :].T, b3[cols][None, :]], axis=0
        ).astype(np.float32)

        hank = np.zeros((128, 2 * NJB * L), np.float32)
        for b in range(NJB):
            for ci in range(2):
                c = 2 * m + ci
                # H[p, e] = x[e - 128*b - p, c] (0 when index < 0)
                w = np.lib.stride_tricks.sliding_window_view(xpad[:, c], L)
                rows = PAD - 128 * b - np.arange(128)
                kk = 2 * b + ci
                hank[:, kk * L : (kk + 1) * L] = w[rows]
        params = np.zeros((H + 1, 608), np.float32)
        params[:H, 0:L] = relrep
        params[:H, 512] = a1
        params[:H, 513] = c1
        params[:H, 514] = c2
        params[:H, 515:547] = W2.T
        params[: H + 1, 547:579] = w3b
        in_maps.append({"params": params, "hank": hank})
    return in_maps


def kernel(**inputs) -> np.ndarray:
    if "nc" not in _CACHE:
        _CACHE["nc"] = _build_module()
    nc = _CACHE["nc"]
    in_maps = _host_prep(inputs)
    res = run_bass_kernel_spmd(nc, in_maps, list(range(NCORES)))
    partial = np.zeros((COUT, L), np.float64)
    for r in res.results:
        partial += r["out"].astype(np.float64)
    return partial.T.astype(np.float32)


# revision 39
# speedup vs baseline: 1.1604x; 1.1604x over previous
"""CKConv (SIREN continuous-kernel conv) Trainium2 Bass kernel.

Math: the reference evaluates a SIREN net at rel[e,s] = t[s] - t_eval[e],
masks causally (rel <= 0), and contracts with x:
    out[e,g] = sum_{s<=e, c} K(rel[e,s])[g,c] * x[s,c]
Both t and t_eval are arange(512)/512, so rel[e,s] = (s-e)/512 exactly in
fp32 -- it depends only on the lag j = e - s in [0, 511].  The net therefore
only needs evaluation at 512 distinct inputs rel_j = -j/512, and the output
is a causal Toeplitz conv:
    out[e] = sum_{j=0}^{e} K'[j] @ x[e-j],   K'[j] in R^{16x16}.

Sharding: 8 cores split the contraction by input channel: core m owns
channels {2m, 2m+1} x all 4 lag blocks of 128.  Host builds Hankel tiles
H[(jb,ci)][p, e] = xpad[e - 128*jb - p, c] (pure data movement of x), sums
the per-core partial (16, 512) outputs and transposes -> (512, 16).

Per-core device program (v2 -- full-width layouts + concurrent PE tiles):
  * "v-layout": partition p = 32*jg + i packs 4 lag-groups x 32 hidden units
    so DVE/ACT stages run on all 128 partitions, and layers 2/3 run as 4
    concurrent 32x32 tile_position matmuls.
  * conv: 8 matmuls (4 lag blocks x 2 channels) at 4 PSUM col-groups, two
    accumulation rounds; partial sums combined with 3 DVE adds.

sin(x) via explicit range reduction (magic-number round-to-nearest):
    u = arg/(2pi) ; k = (u + 1.5*2^23) - 1.5*2^23 ; sin(arg) = Sin(2pi*(u-k))
"""

import numpy as np

import concourse.mybir as mybir
import concourse.tile as tile
from concourse import bacc
from concourse.bass_utils import run_bass_kernel_spmd

F32 = mybir.dt.float32
L = 512          # sequence length == L_eval
CIN = 16
COUT = 16
H = 32           # SIREN hidden
OMEGA = 32.5
NCORES = 8
NJB = 4          # lag blocks of 128
PAD = 512        # zero padding rows in front of x for the Hankel build
TWO_PI = 2.0 * np.pi
MAGIC = float(1.5 * 2.0**23)  # fp32 add/sub rounds to nearest integer

# packed param layout (128, PCOLS), partition p = 32*jg + i
P_REL = 0      # [:, 0:128]   relv[p, jj] = rel[128*jg + jj]
P_A1 = 128     # [:, 128]     A1[i] tiled x4
P_C1 = 129     # [:, 129]     C1[i] tiled x4
P_C2 = 130     # [:, 130]     C2[i] tiled x4
P_W2 = 131     # [:, 131:163] w2v[32jg+i, o] = W2[o, i]  (tiled x4)
P_W3 = 163     # [:, 163:195] w3v[32b+o, m] = W3[colsel[m], o]  (tiled x4)
P_B3 = 195     # [:, 195:227] b3v[p, m] = b3[colsel[m]]  (bcast)
PCOLS = 227

_CACHE = {}


def _build_module():
    # Bacc (not raw Bass): its compile() splits multi-sem sync waits into
    # event-semaphore instructions -- walrus allows only 1 wait per inst.
    nc = bacc.Bacc("TRN2", target_bir_lowering=False, debug=False)

    params_d = nc.dram_tensor("params", [128, PCOLS], F32, kind="ExternalInput")
    # Hankel tiles packed along free dim: chunk k = 2*jb + ci at cols
    # [k*L, (k+1)*L); H_k[p, e] = xpad[e - 128*jb - p, c]
    hank_d = nc.dram_tensor("hank", [128, 2 * NJB * L], F32, kind="ExternalInput")
    out_d = nc.dram_tensor("out", [COUT, L], F32, kind="ExternalOutput")

    with tile.TileContext(nc) as tc:
        with (
            tc.tile_pool(name="sb", bufs=1) as sb,
            tc.tile_pool(name="ps", bufs=1, space="PSUM") as ps,
            tc.tile_pool(name="ps2", bufs=2, space="PSUM") as ps2,
            tc.tile_pool(name="ps4", bufs=1, space="PSUM") as ps4,
        ):
            # Prewarm the Sin ACT table set so its ~2.7us load overlaps DMAs.
            warm = sb.tile([1, 1], F32)
            nc.vector.memset(warm[:], 0.0)
            nc.scalar.activation(warm[:], warm[:], mybir.ActivationFunctionType.Sin)

            pt = sb.tile([128, PCOLS], F32)
            nc.sync.dma_start(pt[:], params_d[:])
            ht = sb.tile([128, 2 * NJB * L], F32)
            nc.sync.dma_start(ht[:], hank_d[:])

            relv = pt[:, P_REL : P_REL + 128]
            a1 = pt[:, P_A1 : P_A1 + 1]
            c1 = pt[:, P_C1 : P_C1 + 1]
            c2 = pt[:, P_C2 : P_C2 + 1]
            w2v = pt[:, P_W2 : P_W2 + H]
            w3v = pt[:, P_W3 : P_W3 + 2 * COUT]
            b3v = pt[:, P_B3 : P_B3 + 2 * COUT]

            # ---- SIREN layer 1 (v-layout, 128 partitions)
            u1 = sb.tile([128, 128], F32)
            nc.vector.tensor_scalar(
                u1[:], relv, a1, c1, mybir.AluOpType.mult, mybir.AluOpType.add
            )
            k1 = sb.tile([128, 128], F32)
            nc.vector.tensor_scalar(
                k1[:], u1[:], MAGIC, MAGIC,
                mybir.AluOpType.add, mybir.AluOpType.subtract,
            )
            nc.vector.tensor_sub(u1[:], u1[:], k1[:])
            h1 = sb.tile([128, 128], F32)
            nc.scalar.activation(
                h1[:], u1[:], mybir.ActivationFunctionType.Sin, scale=TWO_PI
            )

            # ---- SIREN layer 2: 4 concurrent 32x32 tile_position matmuls,
            # output directly in v-layout PSUM (128, 128)
            mm2 = ps.tile([128, 128], F32)
            for jg in range(NJB):
                s = slice(32 * jg, 32 * jg + 32)
                nc.tensor.matmul(
                    mm2[s, :], w2v[s, :], h1[s, :],
                    start=True, stop=True, tile_position=(32 * jg, 32 * jg),
                )
            u2 = sb.tile([128, 128], F32)
            nc.vector.tensor_scalar(
                u2[:], mm2[:], float(OMEGA / TWO_PI), c2,
                mybir.AluOpType.mult, mybir.AluOpType.add,
            )
            k2 = sb.tile([128, 128], F32)
            nc.vector.tensor_scalar(
                k2[:], u2[:], MAGIC, MAGIC,
                mybir.AluOpType.add, mybir.AluOpType.subtract,
            )
            nc.vector.tensor_sub(u2[:], u2[:], k2[:])
            h2 = sb.tile([128, 128], F32)
            nc.scalar.activation(
                h2[:], u2[:], mybir.ActivationFunctionType.Sin, scale=TWO_PI
            )

            # ---- layer 3: K[j, m] per lag block b -- 4 concurrent matmuls
            # (row groups), then +b3 while copying PSUM -> SBUF
            ksb = sb.tile([128, NJB * 2 * COUT], F32)
            for b in range(NJB):
                s = slice(32 * b, 32 * b + 32)
                ktp = ps2.tile([128, 2 * COUT], F32)
                nc.tensor.matmul(
                    ktp[:], h2[s, :], w3v[s, :],
                    start=True, stop=True, tile_position=(32 * b, 0),
                )
                nc.vector.tensor_add(
                    ksb[:, b * 2 * COUT : (b + 1) * 2 * COUT], ktp[:], b3v
                )

            # ---- causal conv: chunk (jb, ci) -> PSUM col-group jb, round ci
            # one PSUM tile per col group so accumulation groups stay 1/bank
            Vs = [
                ps4.tile([128, L], F32, name=f"V{b}", tag=f"V{b}")
                for b in range(NJB)
            ]
            for ci in range(2):
                for b in range(NJB):
                    lhs = ksb[:, b * 2 * COUT + ci * COUT
                              : b * 2 * COUT + (ci + 1) * COUT]
                    rhs = ht[:, (b * 2 + ci) * L : (b * 2 + ci + 1) * L]
                    nc.tensor.matmul(
                        Vs[b][32 * b : 32 * b + COUT, :], lhs, rhs,
                        start=(ci == 0), stop=(ci == 1),
                        tile_position=(0, 32 * b),
                    )

            # combine the 4 col-group partials: out = sum_b Vs[b][32b:32b+16]
            # (DVE may read at most one PSUM operand per instruction)
            th = sb.tile([COUT, L], F32)
            nc.vector.tensor_copy(th[:], Vs[0][0:COUT, :])
            nc.vector.tensor_add(th[:], th[:], Vs[1][32 : 32 + COUT, :])
            nc.vector.tensor_add(th[:], th[:], Vs[2][64 : 64 + COUT, :])
            out_sb = sb.tile([COUT, L], F32)
            nc.vector.tensor_add(out_sb[:], th[:], Vs[3][96 : 96 + COUT, :])
            nc.sync.dma_start(out_d[:], out_sb[:])

    nc.compile()
    return nc


def _host_prep(inputs):
    """Fold params and build per-core in_maps (all fp32 numpy)."""
    x = np.asarray(inputs["x"], np.float32)
    t = np.asarray(inputs["t"], np.float32)
    t_eval = np.asarray(inputs["t_eval"], np.float32)
    v1 = np.asarray(inputs["v1"], np.float32)
    g1 = np.asarray(inputs["g1"], np.float32)
    b1 = np.asarray(inputs["b1"], np.float32)
    v2 = np.asarray(inputs["v2"], np.float32)
    g2 = np.asarray(inputs["g2"], np.float32)
    b2 = np.asarray(inputs["b2"], np.float32)
    W3 = np.asarray(inputs["W3"], np.float32)
    b3 = np.asarray(inputs["b3"], np.float32)

    # weight norm (fp32, matching reference)
    W1 = (g1[:, None] * v1 / np.linalg.norm(v1, axis=1, keepdims=True))[:, 0]
    W2 = g2[:, None] * v2 / np.linalg.norm(v2, axis=1, keepdims=True)

    # rel_j = t[0] - t_eval[j]  (== -j/512 exactly on the arange grid)
    rel = (np.float32(t[0]) - t_eval).astype(np.float32)

    a1 = (np.float64(OMEGA) * W1.astype(np.float64) / TWO_PI).astype(np.float32)
    c1 = (np.float64(OMEGA) * b1.astype(np.float64) / TWO_PI).astype(np.float32)
    c2 = (np.float64(OMEGA) * b2.astype(np.float64) / TWO_PI).astype(np.float32)

    xpad = np.zeros((PAD + L, CIN), np.float32)
    xpad[PAD:] = x

    # shared parts of the packed params (128, PCOLS)
    base = np.zeros((128, PCOLS), np.float32)
    base[:, P_REL : P_REL + 128] = np.repeat(rel.reshape(NJB, 128), H, axis=0)
    base[:, P_A1] = np.tile(a1, NJB)
    base[:, P_C1] = np.tile(c1, NJB)
    base[:, P_C2] = np.tile(c2, NJB)
    base[:, P_W2 : P_W2 + H] = np.tile(W2.T, (NJB, 1))

    in_maps = []
    for m in range(NCORES):
        cols = []
        for ci in range(2):
            c = 2 * m + ci
            cols.extend(g * CIN + c for g in range(COUT))
        params = base.copy()
        params[:, P_W3 : P_W3 + 2 * COUT] = np.tile(W3[cols, :].T, (NJB, 1))
        params[:, P_B3 : P_B3 + 2 * COUT] = np.broadcast_to(b3[cols], (128, 2 * COUT))

        hank = np.zeros((128, 2 * NJB * L), np.float32)
        for b in range(NJB):
            for ci in range(2):
                c = 2 * m + ci
                # H[p, e] = x[e - 128*b - p, c] (0 when index < 0)
                w = np.lib.stride_tricks.sliding_window_view(xpad[:, c], L)
                rows = PAD - 128 * b - np.arange(128)
                kk = 2 * b + ci
                hank[:, kk * L : (kk + 1) * L] = w[rows]
        in_maps.append({"params": params, "hank": hank})
    return in_maps


def kernel(**inputs) -> np.ndarray:
    if "nc" not in _CACHE:
        _CACHE["nc"] = _build_module()
    nc = _CACHE["nc"]
    in_maps = _host_prep(inputs)
    res = run_bass_kernel_spmd(nc, in_maps, list(range(NCORES)))
    partial = np.zeros((COUT, L), np.float64)
    for r in res.results:
        partial += r["out"].astype(np.float64)
    return partial.T.astype(np.float32)
